# revision 21
# baseline (speedup 1.0000x reference)
"""Trainium2 Bass kernel for the FCBlock weight-transform + matmul problem.

Math (per reference):
    W_i = per-head 3x3 conv over W.reshape(4, 1024, 4096) + conv_b
          + sigmoid(sk_wt) * W            (per-head scalars)
    out  = inp @ W_i.T                    (inp: [2, 2048, 4096])

Strategy: tensor-parallel shard of W along fout across 8 NeuronCores
(512 rows each).  The host pre-slices W^T with the conv halo (zero-padded
at head boundaries and fin edges), packs it into the SBUF staging layout,
builds the tiny banded conv matrices from conv_w^T/sigmoid(sk_wt), and
pre-casts everything to bf16.  On each core:
  - stage the whole W^T shard in SBUF (big DMAs issued ahead of the input
    transposes on the same HWDGE ring so they are not starved),
  - run the weight transform as PE band-matmuls accumulating in PSUM,
    which directly yields W_i^T (fin on partitions) - no transposes;
    the conv bias is added during the PSUM->SBUF copy,
  - stream inp via X-bar DMA-transpose (bf16) directly into fin-major
    layout, and run the main matmul in bf16 with fp32 PSUM accumulation.
Output is sharded on fout; the host concatenates.
"""

import numpy as np

import concourse.mybir as mybir
import concourse.tile as tile
from concourse import bacc
from concourse.bass_utils import run_bass_kernel_spmd

F32 = mybir.dt.float32
BF16 = mybir.dt.bfloat16

NCORES = 8
NUM_HEADS = 4
TOK = 4096          # 2 * 2048 tokens
FIN = 4096
FOUT = 4096
FSH = FOUT // NCORES  # 512 fout rows per core
SUP = 512           # token superblock (one transpose-DMA each)


def build_program(tok=TOK, fin=FIN):
    """Build the per-core SPMD program.

    tok/fin are parameters so a mini variant can be compiled quickly for
    validation; the graded path always uses the full sizes.
    """
    assert tok % SUP == 0 and fin % 128 == 0
    n_sup = tok // SUP           # 512-token superblocks
    n_k = fin // 128             # 128-deep contraction blocks / T windows

    nc = bacc.Bacc(None, target_bir_lowering=False)

    xb = nc.declare_dram_parameter("xb", [tok, fin], BF16, isOutput=False)
    wts = nc.declare_dram_parameter("wts", [128, n_k, FSH + 2], BF16,
                                    isOutput=False)
    hts = nc.declare_dram_parameter("hts", [8, n_k, FSH], BF16,
                                    isOutput=False)
    cbnd = nc.declare_dram_parameter("cbnd", [128, 4, 128], BF16,
                                     isOutput=False)
    out = nc.declare_dram_parameter("o", [tok, FSH], BF16, isOutput=True)

    # window-chunk plan: small leading chunks so the transform's first
    # matmuls start as soon as ~260KB has landed; chunks alternate between
    # the sync and scalar DMA rings so staging runs at ~2x one ring's
    # bandwidth.  Total staging DMAs stay <= 9: the shared sem pool has
    # ~12 lanes and every staging DMA must get a lane that is not gated
    # on long-lived waiters (a recycled lane can queue a chunk behind a
    # 15us transpose, stalling the PE for >20us).
    cuts = [0]
    for sz in [2, 2, 4] + [8] * 64:
        cuts.append(min(cuts[-1] + sz, n_k))
        if cuts[-1] == n_k:
            break
    chunks = list(zip(cuts[:-1], cuts[1:]))
    groups = chunks

    with tile.TileContext(nc) as tc:
        with (
            tc.tile_pool(name="const", bufs=1) as const,
            tc.tile_pool(name="wtpool", bufs=1) as wtpool,
            tc.tile_pool(name="xt", bufs=2) as xtp,
            tc.tile_pool(name="osb", bufs=6) as osbp,
            tc.tile_pool(name="ps", bufs=8, space="PSUM") as ps,
        ):
            # ---- input staging: everything on the sync ring, in strict
            # need-order (a single ring sustains ~270GB/s > the transform's
            # ~150GB/s consumption; the ring's issue->data latency puts a
            # ~12.5us floor on the first window regardless of chunk size).
            # Putting a second ring (scalar) to work backfires: the tile
            # scheduler chains its chunks behind the first 15us input
            # transpose.  The input transposes queue on sync strictly after
            # staging; the first superblock's transpose is split so m-tile
            # 0's tokens land before the weight transform finishes.
            # (chunk0 strictly FIRST: leading with the small cbnd tensor
            # measurably delays chunk0's arrival by ~5us)
            wst = const.tile([128, n_k, FSH + 2], BF16, tag="wst")
            hst = const.tile([8, n_k, FSH], BF16, tag="hst")
            cbnd_sb = const.tile([128, 4, 128], BF16)
            nc.sync.dma_start(out=wst[:, chunks[0][0]:chunks[0][1], :],
                              in_=wts[:, chunks[0][0]:chunks[0][1], :])
            nc.sync.dma_start(out=cbnd_sb[:], in_=cbnd[:])
            nc.sync.dma_start(out=hst[:], in_=hts[:])
            for g0, g1 in chunks[1:]:
                nc.sync.dma_start(out=wst[:, g0:g1, :], in_=wts[:, g0:g1, :])

            # (No PE warm-up: the PE's post-idle ramp (~8 matmuls at 2x
            # interval) and the Tensor queue's first instruction-buffer
            # refill (~3.5us) both hide inside the staging-data wait; warm-
            # up matmuls push the refill into the busy window and lose.)

            wt = wtpool.tile([128, n_k, FSH], BF16)        # W_i^T, fin-major

            # ---- phase T: weight transform straight into W_i^T ------------
            # window groups with tap-outer ordering: amortizes stationary
            # switches (the [6,...] halo tiles break LDWEIGHTS pull-ahead)
            for q, q1 in groups:
                pws = [ps.tile([128, FSH], F32, tag="ps", name=f"pw{q}_{j}")
                       for j in range(q1 - q)]
                for a in range(3):
                    for j, pw in enumerate(pws):
                        nc.tensor.matmul(
                            pw[:], cbnd_sb[:, a, :],
                            wst[:, q + j, a:a + FSH],
                            start=(a == 0), stop=False)
                for j, pw in enumerate(pws):
                    # halo rows 0-5 + conv-bias hi/lo rows 6-7 (stationary
                    # rows 6-7 are all-ones, so this also adds the bias)
                    nc.tensor.matmul(pw[:], cbnd_sb[0:8, 3, :],
                                     hst[:, q + j, :],
                                     start=False, stop=True)
                for j, pw in enumerate(pws):
                    i = q + j
                    # PSUM -> SBUF, cast to bf16
                    if j % 2 == 0:
                        nc.scalar.copy(out=wt[:, i, :], in_=pw[:])
                    else:
                        nc.vector.tensor_copy(out=wt[:, i, :], in_=pw[:])

            # ---- phase M: main matmul ---------------------------------
            # output in bf16 (host upcasts); the last m-tile accumulates in
            # two 256-col half-groups so half A's copy+DMA overlaps half B's
            # matmuls instead of sitting fully after the final matmul.
            for t in range(n_sup):
                xt = xtp.tile([128, n_k, SUP], BF16, tag="xt")
                if t == 0:
                    nc.sync.dma_start(out=xt[:, :, 0:128],
                                      in_=xb[0:128, :], transpose=True)
                    nc.sync.dma_start(out=xt[:, :, 128:SUP],
                                      in_=xb[128:SUP, :], transpose=True)
                else:
                    nc.sync.dma_start(out=xt[:],
                                      in_=xb[SUP * t:SUP * t + SUP, :],
                                      transpose=True)
                for m in range(SUP // 128):
                    last = (t == n_sup - 1) and (m == SUP // 128 - 1)
                    po = ps.tile([128, FSH], F32, tag="ps")
                    row0 = SUP * t + 128 * m
                    if not last:
                        for k in range(n_k):
                            nc.tensor.matmul(po[:],
                                             xt[:, k, 128 * m:128 * m + 128],
                                             wt[:, k, :],
                                             start=(k == 0),
                                             stop=(k == n_k - 1))
                        ob = osbp.tile([128, FSH], BF16, tag="ob")
                        if m % 2 == 0:
                            nc.scalar.copy(out=ob[:], in_=po[:])
                        else:
                            nc.vector.tensor_copy(out=ob[:], in_=po[:])
                        nc.scalar.dma_start(out=out[row0:row0 + 128, :],
                                            in_=ob[:])
                    else:
                        # split the final m-tile into two 256-col halves in
                        # separate psum banks: half A's copy+DMA overlaps
                        # half B's matmuls instead of trailing the last one
                        half = FSH // 2
                        po2 = ps.tile([128, FSH], F32, tag="ps",
                                      name="po_last_b")
                        for h, pt in ((0, po), (1, po2)):
                            cs = slice(half * h, half * h + half)
                            for k in range(n_k):
                                nc.tensor.matmul(
                                    pt[:, 0:half],
                                    xt[:, k, 128 * m:128 * m + 128],
                                    wt[:, k, cs],
                                    start=(k == 0),
                                    stop=(k == n_k - 1))
                            ob = osbp.tile([128, half], BF16, tag="obh",
                                           name=f"obh{h}")
                            if h == 0:
                                nc.scalar.copy(out=ob[:], in_=pt[:, 0:half])
                                nc.scalar.dma_start(
                                    out=out[row0:row0 + 128, cs], in_=ob[:])
                            else:
                                nc.vector.tensor_copy(out=ob[:],
                                                      in_=pt[:, 0:half])
                                nc.sync.dma_start(
                                    out=out[row0:row0 + 128, cs], in_=ob[:])

    nc.compile()
    return nc


def shard_inputs(inp, W, conv_w, conv_b, sk_wt, fin=FIN):
    """Build the 8 per-core input maps: W^T fout-shard with conv halo,
    packed into the on-device staging layout, plus host-built band
    matrices (conv taps transposed, sigmoid residual folded in)."""
    bf = mybir.dt.np(BF16)
    tok = inp.size // fin
    xb = np.ascontiguousarray(
        inp.reshape(tok, fin)).astype(np.float32).astype(bf)
    W = np.asarray(W, dtype=np.float32)
    conv_w = np.asarray(conv_w, dtype=np.float32)
    hsz = W.shape[0] // NUM_HEADS  # rows per head
    n_k = fin // 128
    in_maps = []
    for c in range(NCORES):
        gr0 = c * FSH
        h = (gr0 // hsz) % NUM_HEADS
        # whal[R, C] = W[gr0-1+R, C-1], zero outside the head / fin range
        whal = np.zeros((FSH + 2, fin + 2), dtype=np.float32)
        lo = max(gr0 - 1, h * hsz)
        hi = min(gr0 + FSH + 1, (h + 1) * hsz)
        whal[lo - (gr0 - 1):hi - (gr0 - 1), 1:fin + 1] = W[lo:hi, :fin]
        # staged W^T: wts[k, i, c] = whal[c, 128i + k]
        wtslab = np.ascontiguousarray(whal.T)          # [fin+2, FSH+2]
        wts = np.ascontiguousarray(
            wtslab[:n_k * 128].reshape(n_k, 128, FSH + 2)
            .transpose(1, 0, 2))                       # [128, n_k, FSH+2]
        # rows 0-5: conv halo; rows 6-7: conv-bias hi/lo (the matching
        # stationary rows are all-ones, so the halo matmul adds the bias)
        hts = np.empty((8, n_k, FSH), dtype=np.float32)
        for a in range(2):
            for b in range(3):
                hts[3 * a + b] = wtslab[128 + a:128 * n_k + 128 + a:128,
                                        b:b + FSH]
        cb_h = np.float32(np.asarray(conv_b)[h])
        cb_hi = np.float32(cb_h.astype(bf))
        hts[6] = cb_hi
        hts[7] = np.float32(cb_h - cb_hi)
        # band matrices (conv taps transposed); sigmoid residual on the
        # (a=1, d=1) diagonal; halo matrix in cbnd[:, 3, :]
        cwt = conv_w[h].reshape(3, 3).T
        sig = float(1.0 / (1.0 + np.exp(-np.float64(
            np.asarray(sk_wt, dtype=np.float32)[h].reshape(())))))
        cbnd = np.zeros((128, 4, 128), dtype=np.float32)
        for a in range(3):
            for d in range(3):
                cbnd[:, a, :] += np.eye(128, k=-d, dtype=np.float32) \
                    * cwt[d, a]
        cbnd[:, 1, :] += np.eye(128, k=-1, dtype=np.float32) * sig
        h8 = np.zeros((8, 128), dtype=np.float32)
        for b in range(3):
            h8[b, 127] = cwt[1, b]
            h8[3 + b, 127] = cwt[2, b]
            h8[b, 126] = cwt[2, b]
        h8[1, 127] += sig
        h8[6, :] = 1.0
        h8[7, :] = 1.0
        cbnd[0:8, 3, :] = h8
        in_maps.append({"xb": xb, "wts": wts.astype(bf),
                        "hts": hts.astype(bf),
                        "cbnd": cbnd.astype(bf)})
    return in_maps


_PROGRAM_CACHE = {}


def _get_program(tok, fin):
    key = (tok, fin)
    if key not in _PROGRAM_CACHE:
        _PROGRAM_CACHE[key] = build_program(tok, fin)
    return _PROGRAM_CACHE[key]


def kernel(inp, W, conv_w, conv_b, sk_wt):
    nc = _get_program(TOK, FIN)
    in_maps = shard_inputs(inp, W, conv_w, conv_b, sk_wt)
    res = run_bass_kernel_spmd(nc, in_maps, list(range(NCORES)))
    shards = [np.asarray(res.results[c]["o"]).astype(np.float32)
              .reshape(2, TOK // 2, FSH) for c in range(NCORES)]
    return np.ascontiguousarray(np.concatenate(shards, axis=-1))



# revision 23
# speedup vs baseline: 1.0080x; 1.0080x over previous
"""Trainium2 Bass kernel for the FCBlock weight-transform + matmul problem.

Math (per reference):
    W_i = per-head 3x3 conv over W.reshape(4, 1024, 4096) + conv_b
          + sigmoid(sk_wt) * W            (per-head scalars)
    out  = inp @ W_i.T                    (inp: [2, 2048, 4096])

Strategy: tensor-parallel shard of W along fout across 8 NeuronCores
(512 rows each).  The host pre-slices W^T with the conv halo (zero-padded
at head boundaries and fin edges), packs it into the SBUF staging layout,
builds the tiny banded conv matrices from conv_w^T/sigmoid(sk_wt), and
pre-casts everything to bf16.  On each core:
  - stage the whole W^T shard in SBUF (big DMAs issued ahead of the input
    transposes on the same HWDGE ring so they are not starved),
  - run the weight transform as PE band-matmuls accumulating in PSUM,
    which directly yields W_i^T (fin on partitions) - no transposes;
    the conv bias is added during the PSUM->SBUF copy,
  - stream inp via X-bar DMA-transpose (bf16) directly into fin-major
    layout, and run the main matmul in bf16 with fp32 PSUM accumulation.
Output is sharded on fout; the host concatenates.
"""

import numpy as np

import concourse.mybir as mybir
import concourse.tile as tile
from concourse import bacc
from concourse.bass_utils import run_bass_kernel_spmd

F32 = mybir.dt.float32
BF16 = mybir.dt.bfloat16

NCORES = 8
NUM_HEADS = 4
TOK = 4096          # 2 * 2048 tokens
FIN = 4096
FOUT = 4096
FSH = FOUT // NCORES  # 512 fout rows per core
SUP = 512           # token superblock (one transpose-DMA each)


def build_program(tok=TOK, fin=FIN):
    """Build the per-core SPMD program.

    tok/fin are parameters so a mini variant can be compiled quickly for
    validation; the graded path always uses the full sizes.
    """
    assert tok % SUP == 0 and fin % 128 == 0
    n_sup = tok // SUP           # 512-token superblocks
    n_k = fin // 128             # 128-deep contraction blocks / T windows

    nc = bacc.Bacc(None, target_bir_lowering=False)

    xb = nc.declare_dram_parameter("xb", [tok, fin], BF16, isOutput=False)
    wts = nc.declare_dram_parameter("wts", [128, n_k, FSH + 2], BF16,
                                    isOutput=False)
    hts = nc.declare_dram_parameter("hts", [8, n_k, FSH], BF16,
                                    isOutput=False)
    cbnd = nc.declare_dram_parameter("cbnd", [128, 4, 128], BF16,
                                     isOutput=False)
    out = nc.declare_dram_parameter("o", [tok, FSH], BF16, isOutput=True)

    # window-chunk plan: small leading chunks so the transform's first
    # matmuls start as soon as ~260KB has landed; chunks alternate between
    # the sync and scalar DMA rings so staging runs at ~2x one ring's
    # bandwidth.  Total staging DMAs stay <= 9: the shared sem pool has
    # ~12 lanes and every staging DMA must get a lane that is not gated
    # on long-lived waiters (a recycled lane can queue a chunk behind a
    # 15us transpose, stalling the PE for >20us).
    cuts = [0]
    for sz in [2, 4, 4] + [8] * 64:
        cuts.append(min(cuts[-1] + sz, n_k))
        if cuts[-1] == n_k:
            break
    chunks = list(zip(cuts[:-1], cuts[1:]))
    groups = chunks

    with tile.TileContext(nc) as tc:
        with (
            tc.tile_pool(name="const", bufs=1) as const,
            tc.tile_pool(name="wtpool", bufs=1) as wtpool,
            tc.tile_pool(name="xt", bufs=2) as xtp,
            tc.tile_pool(name="osb", bufs=6) as osbp,
            tc.tile_pool(name="ps", bufs=8, space="PSUM") as ps,
        ):
            # ---- input staging: everything on the sync ring, in strict
            # need-order (a single ring sustains ~270GB/s > the transform's
            # ~150GB/s consumption; the ring's issue->data latency puts a
            # ~12.5us floor on the first window regardless of chunk size).
            # Putting a second ring (scalar) to work backfires: the tile
            # scheduler chains its chunks behind the first 15us input
            # transpose.  The input transposes queue on sync strictly after
            # staging; the first superblock's transpose is split so m-tile
            # 0's tokens land before the weight transform finishes.
            # (tiny chunk0 strictly FIRST -- the first transform matmul then
            # starts at ~10.4us instead of 12.6; a big tensor first delays
            # everything behind its full transfer.  hts is deferred past
            # chunk1 because the first halo matmul runs ~3us of (ramped)
            # tap matmuls after the first window lands.)
            wst = const.tile([128, n_k, FSH + 2], BF16, tag="wst")
            hst = const.tile([8, n_k, FSH], BF16, tag="hst")
            cbnd_sb = const.tile([128, 4, 128], BF16)
            nc.sync.dma_start(out=wst[:, chunks[0][0]:chunks[0][1], :],
                              in_=wts[:, chunks[0][0]:chunks[0][1], :])
            nc.sync.dma_start(out=cbnd_sb[:], in_=cbnd[:])
            for i, (g0, g1) in enumerate(chunks[1:]):
                nc.sync.dma_start(out=wst[:, g0:g1, :], in_=wts[:, g0:g1, :])
                if i == 0:
                    nc.sync.dma_start(out=hst[:], in_=hts[:])

            # (No PE warm-up: the PE's post-idle ramp (~8 matmuls at 2x
            # interval) and the Tensor queue's first instruction-buffer
            # refill (~3.5us) both hide inside the staging-data wait; warm-
            # up matmuls push the refill into the busy window and lose.)

            wt = wtpool.tile([128, n_k, FSH], BF16)        # W_i^T, fin-major

            # ---- phase T: weight transform straight into W_i^T ------------
            # window groups with tap-outer ordering: amortizes stationary
            # switches (the [6,...] halo tiles break LDWEIGHTS pull-ahead)
            for q, q1 in groups:
                pws = [ps.tile([128, FSH], F32, tag="ps", name=f"pw{q}_{j}")
                       for j in range(q1 - q)]
                for a in range(3):
                    for j, pw in enumerate(pws):
                        nc.tensor.matmul(
                            pw[:], cbnd_sb[:, a, :],
                            wst[:, q + j, a:a + FSH],
                            start=(a == 0), stop=False)
                for j, pw in enumerate(pws):
                    # halo rows 0-5 + conv-bias hi/lo rows 6-7 (stationary
                    # rows 6-7 are all-ones, so this also adds the bias)
                    nc.tensor.matmul(pw[:], cbnd_sb[0:8, 3, :],
                                     hst[:, q + j, :],
                                     start=False, stop=True)
                for j, pw in enumerate(pws):
                    i = q + j
                    # PSUM -> SBUF, cast to bf16
                    if j % 2 == 0:
                        nc.scalar.copy(out=wt[:, i, :], in_=pw[:])
                    else:
                        nc.vector.tensor_copy(out=wt[:, i, :], in_=pw[:])

            # ---- phase M: main matmul ---------------------------------
            # output in bf16 (host upcasts); the last m-tile accumulates in
            # two 256-col half-groups so half A's copy+DMA overlaps half B's
            # matmuls instead of sitting fully after the final matmul.
            for t in range(n_sup):
                xt = xtp.tile([128, n_k, SUP], BF16, tag="xt")
                if t == 0:
                    nc.sync.dma_start(out=xt[:, :, 0:128],
                                      in_=xb[0:128, :], transpose=True)
                    nc.sync.dma_start(out=xt[:, :, 128:SUP],
                                      in_=xb[128:SUP, :], transpose=True)
                else:
                    nc.sync.dma_start(out=xt[:],
                                      in_=xb[SUP * t:SUP * t + SUP, :],
                                      transpose=True)
                for m in range(SUP // 128):
                    last = (t == n_sup - 1) and (m == SUP // 128 - 1)
                    po = ps.tile([128, FSH], F32, tag="ps")
                    row0 = SUP * t + 128 * m
                    if not last:
                        for k in range(n_k):
                            nc.tensor.matmul(po[:],
                                             xt[:, k, 128 * m:128 * m + 128],
                                             wt[:, k, :],
                                             start=(k == 0),
                                             stop=(k == n_k - 1))
                        ob = osbp.tile([128, FSH], BF16, tag="ob")
                        if m % 2 == 0:
                            nc.scalar.copy(out=ob[:], in_=po[:])
                        else:
                            nc.vector.tensor_copy(out=ob[:], in_=po[:])
                        nc.scalar.dma_start(out=out[row0:row0 + 128, :],
                                            in_=ob[:])
                    else:
                        # split the final m-tile into two 256-col halves in
                        # separate psum banks: half A's copy+DMA overlaps
                        # half B's matmuls instead of trailing the last one
                        half = FSH // 2
                        po2 = ps.tile([128, FSH], F32, tag="ps",
                                      name="po_last_b")
                        for h, pt in ((0, po), (1, po2)):
                            cs = slice(half * h, half * h + half)
                            for k in range(n_k):
                                nc.tensor.matmul(
                                    pt[:, 0:half],
                                    xt[:, k, 128 * m:128 * m + 128],
                                    wt[:, k, cs],
                                    start=(k == 0),
                                    stop=(k == n_k - 1))
                            ob = osbp.tile([128, half], BF16, tag="obh",
                                           name=f"obh{h}")
                            if h == 0:
                                nc.scalar.copy(out=ob[:], in_=pt[:, 0:half])
                                nc.scalar.dma_start(
                                    out=out[row0:row0 + 128, cs], in_=ob[:])
                            else:
                                nc.vector.tensor_copy(out=ob[:],
                                                      in_=pt[:, 0:half])
                                nc.sync.dma_start(
                                    out=out[row0:row0 + 128, cs], in_=ob[:])

    nc.compile()
    return nc


def shard_inputs(inp, W, conv_w, conv_b, sk_wt, fin=FIN):
    """Build the 8 per-core input maps: W^T fout-shard with conv halo,
    packed into the on-device staging layout, plus host-built band
    matrices (conv taps transposed, sigmoid residual folded in)."""
    bf = mybir.dt.np(BF16)
    tok = inp.size // fin
    xb = np.ascontiguousarray(
        inp.reshape(tok, fin)).astype(np.float32).astype(bf)
    W = np.asarray(W, dtype=np.float32)
    conv_w = np.asarray(conv_w, dtype=np.float32)
    hsz = W.shape[0] // NUM_HEADS  # rows per head
    n_k = fin // 128
    in_maps = []
    for c in range(NCORES):
        gr0 = c * FSH
        h = (gr0 // hsz) % NUM_HEADS
        # whal[R, C] = W[gr0-1+R, C-1], zero outside the head / fin range
        whal = np.zeros((FSH + 2, fin + 2), dtype=np.float32)
        lo = max(gr0 - 1, h * hsz)
        hi = min(gr0 + FSH + 1, (h + 1) * hsz)
        whal[lo - (gr0 - 1):hi - (gr0 - 1), 1:fin + 1] = W[lo:hi, :fin]
        # staged W^T: wts[k, i, c] = whal[c, 128i + k]
        wtslab = np.ascontiguousarray(whal.T)          # [fin+2, FSH+2]
        wts = np.ascontiguousarray(
            wtslab[:n_k * 128].reshape(n_k, 128, FSH + 2)
            .transpose(1, 0, 2))                       # [128, n_k, FSH+2]
        # rows 0-5: conv halo; rows 6-7: conv-bias hi/lo (the matching
        # stationary rows are all-ones, so the halo matmul adds the bias)
        hts = np.empty((8, n_k, FSH), dtype=np.float32)
        for a in range(2):
            for b in range(3):
                hts[3 * a + b] = wtslab[128 + a:128 * n_k + 128 + a:128,
                                        b:b + FSH]
        cb_h = np.float32(np.asarray(conv_b)[h])
        cb_hi = np.float32(cb_h.astype(bf))
        hts[6] = cb_hi
        hts[7] = np.float32(cb_h - cb_hi)
        # band matrices (conv taps transposed); sigmoid residual on the
        # (a=1, d=1) diagonal; halo matrix in cbnd[:, 3, :]
        cwt = conv_w[h].reshape(3, 3).T
        sig = float(1.0 / (1.0 + np.exp(-np.float64(
            np.asarray(sk_wt, dtype=np.float32)[h].reshape(())))))
        cbnd = np.zeros((128, 4, 128), dtype=np.float32)
        for a in range(3):
            for d in range(3):
                cbnd[:, a, :] += np.eye(128, k=-d, dtype=np.float32) \
                    * cwt[d, a]
        cbnd[:, 1, :] += np.eye(128, k=-1, dtype=np.float32) * sig
        h8 = np.zeros((8, 128), dtype=np.float32)
        for b in range(3):
            h8[b, 127] = cwt[1, b]
            h8[3 + b, 127] = cwt[2, b]
            h8[b, 126] = cwt[2, b]
        h8[1, 127] += sig
        h8[6, :] = 1.0
        h8[7, :] = 1.0
        cbnd[0:8, 3, :] = h8
        in_maps.append({"xb": xb, "wts": wts.astype(bf),
                        "hts": hts.astype(bf),
                        "cbnd": cbnd.astype(bf)})
    return in_maps


_PROGRAM_CACHE = {}


def _get_program(tok, fin):
    key = (tok, fin)
    if key not in _PROGRAM_CACHE:
        _PROGRAM_CACHE[key] = build_program(tok, fin)
    return _PROGRAM_CACHE[key]


def kernel(inp, W, conv_w, conv_b, sk_wt):
    nc = _get_program(TOK, FIN)
    in_maps = shard_inputs(inp, W, conv_w, conv_b, sk_wt)
    res = run_bass_kernel_spmd(nc, in_maps, list(range(NCORES)))
    shards = [np.asarray(res.results[c]["o"]).astype(np.float32)
              .reshape(2, TOK // 2, FSH) for c in range(NCORES)]
    return np.ascontiguousarray(np.concatenate(shards, axis=-1))

